# revision 48
# baseline (speedup 1.0000x reference)
"""Trainium2 Bass kernel for nn_MultiHeadAttention_85375359909998.

Causal MHA with (non-standard interleaved) RoPE, fp32 in/out.
  B=2, T=2048, D=1024, H=16, DH=64.

Sharding over 8 NeuronCores: data-parallel over batch (2) x tensor-parallel
over head groups (16 heads -> 4 groups of 4). Each core computes its batch's
QKV projection for its 4 heads, RoPE, causal attention, and a partial output
projection; the host sums the 4 partial projections per batch (the
"all-reduce") and concatenates batches.

Device-side layout notes (per core, heads grouped in pairs):
  - PE operands are fp16; accumulation stays fp32 in PSUM.
  - q/k are produced *transposed* ([dh, t]) by the projection (host passes
    x^T and W^T); RoPE rotate-half is a 128x128 block-diagonal permutation
    matrix applied on the PE; the cos/sin combine is split DVE(muls) /
    GpSimd(add).
  - scores are computed transposed (S^T[s, t]); V carries 64 appended
    ones-columns per head so A@V PSUM rows 64-127 equal the softmax
    denominator; exp eviction on Scalar (the only activation engine) with
    column-trimmed activations on late-diagonal groups; the in-block causal
    triangle is a [128,128] 0/1 mask multiply on GpSimd.
  - engine budget: PE ~116us is the floor; Scalar ~80 (exp only); DVE ~65
    (all PSUM evictions + recip); GpSimd ~45 (SBUF-only ops; it has no PSUM
    port and is ~2.6x slower per element).
  - scheduling: phase-1 (qkv+rope) and output-projection work is split into
    small self-contained "units" pumped one-at-a-time between attention
    score groups, so the PE stays fed through the scalar-heavy late strips;
    all units accumulate in the 2 "a" PSUM banks, scores own 4 "s" banks,
    attention-out accumulators own "o0"/"o1".
  - all DMA issues ride the sync queue except the head-of-kernel x^T /
    cos/sin loads (spread over scalar/gpsimd/vector while those are idle).
  - a warm-up matmul stream covers the initial DMA window so the PE HAM
    clock-gate is released before the first real matmuls.
  - partial projection outputs leave as fp16 (host accumulates in fp32).
"""

import sys
from collections import deque
from contextlib import ExitStack

import numpy as np

try:
    import concourse.bass as bass  # noqa: F401
except ImportError:  # pragma: no cover
    sys.path.insert(0, "/opt/trn_rl_repo")
    import concourse.bass as bass  # noqa: F401

import concourse.tile as tile
from concourse import bacc, mybir
from concourse import bass_utils

B, T, D, H, DH = 2, 2048, 1024, 16, 64
NCORES = 8
GROUPS = 4          # head groups (tensor-parallel dimension)
HPC = H // GROUPS   # 4 heads per core
NPAIR = HPC // 2    # head pairs per core
TC512 = T // 512    # 4
SC128 = T // 128    # 16
KC = D // 128       # 8 contraction chunks for the projections
WARM = 10           # PE warm-up matmuls during the initial DMA wait

f32 = mybir.dt.float32
f16 = mybir.dt.float16
EXP = mybir.ActivationFunctionType.Exp
# adjacent-lane swap within each 32-partition quadrant (rotate-half)
_SWAP_MASK = [i + 1 if i % 2 == 0 else i - 1 for i in range(32)]

_CACHE = {}


def _rope_tables():
    """cos/sin tables, transposed & stacked for the [2*64, t] chunk layout."""
    inv = 1.0 / (10000.0 ** (np.arange(0, DH, 2, dtype=np.float64) / DH))  # 32
    t = np.arange(T, dtype=np.float64)
    freqs = t[:, None] * inv[None, :]                 # [T, 32]
    emb = np.concatenate([freqs, freqs], axis=-1)     # [T, 64]
    cos = np.cos(emb).astype(np.float32).T            # [64, T]
    sin = np.sin(emb).astype(np.float32).T
    csc = np.concatenate([cos, cos], axis=0)          # [128, T]
    csn = np.concatenate([sin, sin], axis=0)
    return (np.ascontiguousarray(csc.astype(np.float16)),
            np.ascontiguousarray(csn.astype(np.float16)))


def _rot_matrix():
    """R.T for rotate_half: (R@v)[2i] = -v[2i+1], (R@v)[2i+1] = v[2i]."""
    R = np.zeros((DH, DH), dtype=np.float32)
    for i in range(DH // 2):
        R[2 * i, 2 * i + 1] = -1.0
        R[2 * i + 1, 2 * i] = 1.0
    R128 = np.zeros((128, 128), dtype=np.float32)
    R128[:DH, :DH] = R
    R128[DH:, DH:] = R
    return np.ascontiguousarray(R128.T)


def _tri_mask():
    """tri[s, c] = 1 if c >= s else 0: keep-mask for diagonal 128-blocks."""
    sp = np.arange(128)[:, None]
    cp = np.arange(512)[None, :]
    return np.ascontiguousarray((cp >= sp).astype(np.float16))


def _emit(nc, tc, d, ctx):
    const = ctx.enter_context(tc.tile_pool(name="const", bufs=1))
    qkp = ctx.enter_context(tc.tile_pool(name="qk", bufs=1))
    vp = ctx.enter_context(tc.tile_pool(name="v", bufs=1))
    att = ctx.enter_context(tc.tile_pool(name="att", bufs=1))
    ptp = ctx.enter_context(tc.tile_pool(name="pt", bufs=6))
    tmp = ctx.enter_context(tc.tile_pool(name="tmp", bufs=6))
    small = ctx.enter_context(tc.tile_pool(name="small", bufs=4))
    stage = ctx.enter_context(tc.tile_pool(name="stage", bufs=1))
    # single PSUM pool, 8 banks: "a" (all phase-1/rope/proj accumulators +
    # warm-up) 2, "s" (scores only — nothing else may rotate through these
    # banks or it serializes against the exp pipeline) 4, "o0"/"o1"
    # (attention-out accumulators) 2
    ps = ctx.enter_context(tc.tile_pool(name="ps", bufs=1, space="PSUM"))

    # ---- constants; DMA issue queues chosen so the head-of-kernel loads
    #      spread across 4 idle queues, first-needed first ----
    warm_t = const.tile([128, 512], f16, tag="warm")
    nc.vector.memset(warm_t[:], 0.0)

    # DMA plan: host packs every input so each tensor is ONE wide tile
    # whose DRAM rows are fully contiguous — each dma_start costs ~0.6us
    # on its issuing sequencer and a ring slot, so instruction count is
    # what matters. 14 input instructions total, all on the two HWDGE
    # queues (sync + scalar); the gpsimd SWDGE path is never used.
    # xT layout: [128, 4 chunks x (8 kc x 512)] — each 512-col t-chunk of
    # all kc blocks is one contiguous 8KB-per-row region = 1 DMA each
    xT2_t = const.tile([128, KC * T], f16, tag="xT2")

    def xs(kc, tcc):
        base = 4096 * tcc + 512 * kc
        return xT2_t[:, base:base + 512]

    wqk2_t = const.tile([128, KC * 512], f16, tag="wqk2")
    wqk_t = [wqk2_t[:, 512 * kc:512 * (kc + 1)] for kc in range(KC)]
    rT_t = const.tile([128, 128], f16, tag="rT")
    csc_t = const.tile([128, T], f16, tag="csc")
    csn_t = const.tile([128, T], f16, tag="csn")
    wv2_t = const.tile([128, KC * 256], f16, tag="wv2")
    wv_t = [wv2_t[:, 256 * kc:256 * (kc + 1)] for kc in range(KC)]
    tri_t = const.tile([128, 128], f16, tag="tri")
    wp2_t = const.tile([128, 2 * D], f16, tag="wp2")
    wp_t = [wp2_t[:, D * kc2:D * (kc2 + 1)] for kc2 in range(2)]

    # first chunk + qk weights in kc 0-3 / 4-7 halves: the projection's
    # kc accumulation loop starts as soon as the first halves land
    nc.sync.dma_start(wqk2_t[:, 0:2048], d["wqk2"][:, 0:2048])
    nc.scalar.dma_start(xT2_t[:, 0:2048], d["xT2"][:, 0:2048])
    nc.sync.dma_start(wqk2_t[:, 2048:4096], d["wqk2"][:, 2048:4096])
    nc.scalar.dma_start(xT2_t[:, 2048:4096], d["xT2"][:, 2048:4096])
    nc.sync.dma_start(rT_t[:], d["rT"][:])
    nc.sync.dma_start(csc_t[:, 0:512], d["csc"][:, 0:512])
    nc.sync.dma_start(csn_t[:, 0:512], d["csn"][:, 0:512])
    nc.sync.dma_start(wv2_t[:], d["wv2"][:])
    nc.scalar.dma_start(xT2_t[:, 4096:8192], d["xT2"][:, 4096:8192])
    nc.sync.dma_start(csc_t[:, 512:2048], d["csc"][:, 512:2048])
    nc.sync.dma_start(csn_t[:, 512:2048], d["csn"][:, 512:2048])
    nc.sync.dma_start(tri_t[:], d["tri"][:, 0:128])
    nc.scalar.dma_start(xT2_t[:, 8192:12288], d["xT2"][:, 8192:12288])
    nc.scalar.dma_start(xT2_t[:, 12288:16384], d["xT2"][:, 12288:16384])
    nc.sync.dma_start(wp2_t[:], d["wp2"][:])

    # ---- persistent activations ----
    qk = [qkp.tile([128, T], f16, tag=f"qk{j}", name=f"qk{j}")
          for j in range(4)]
    # v layout per s-chunk: [V_h0 | ones | V_h1 | ones | ...] so the A@V
    # stationary [128, 128] = [V_h | ones]; the 64 ones columns make PSUM
    # rows 64-127 of the A@V output all equal the softmax denominator.
    v_all = vp.tile([128, SC128 * 512], f16, tag="vall")
    nc.vector.memset(
        v_all.rearrange("p (n c) -> p n c", n=4 * SC128, c=128)[:, :, 64:128],
        1.0)
    attn = [att.tile([128, T], f16, tag=f"at{p}", name=f"at{p}")
            for p in range(NPAIR)]
    # output staging: one wide tile laid out [strip][oc][512] so each
    # strip is one contiguous 8KB-per-row DMA instead of 32 small stores
    obuf = stage.tile([128, KC * T], f16, tag="obuf")
    oc_done = [0] * TC512

    def out_flush(lo, hi):
        nc.sync.dma_start(d["o2"][:, lo:hi], obuf[:, lo:hi])

    # warm-up matmuls: keep the PE array active (HAM clock ungated) while
    # the first x^T / weight DMAs stream in; interleaved "posts" read each
    # just-landed tile so the warm stream is paced WITH the loads instead
    # of draining instantly ahead of them
    def warm_mm(moving=None, n=1):
        for _ in range(n):
            wps = ps.tile([128, 512], f32, tag="a", bufs=2, name="wps")
            src = moving if moving is not None else warm_t[:, 0:256]
            nc.tensor.matmul(wps[:, 0:src.shape[-1]], warm_t[:, 0:128],
                             src, start=True, stop=True)

    warm_mm(n=WARM)
    warm_mm(wqk2_t[:, 0:256])
    warm_mm(n=3)
    warm_mm(rT_t[:, 0:128])
    warm_mm(n=3)
    warm_mm(csc_t[:, 0:256])
    warm_mm(n=3)
    warm_mm(csn_t[:, 0:256])
    warm_mm(n=3)
    warm_mm(wv2_t[:, 0:256])
    warm_mm(n=3)

    # ===== self-contained emission units ================================
    def qk_unit(tcc, jc):
        """One 128-row q-or-k block for t-chunk tcc: proj + RoPE."""
        tsl = slice(512 * tcc, 512 * (tcc + 1))
        pq = ps.tile([128, 512], f32, tag="a", bufs=2, name="pq")
        for kc in range(KC):
            nc.tensor.matmul(pq[:],
                             wqk_t[kc][:, 128 * jc:128 * (jc + 1)],
                             xs(kc, tcc),
                             start=(kc == 0), stop=(kc == KC - 1))
        dst = qk[jc][:, tsl]
        # eviction: Scalar is idle before the first exp, DVE after
        if tcc == 0:
            nc.scalar.copy(dst, pq[:])
        else:
            nc.vector.tensor_copy(dst, pq[:])
        # rotate-half via the PE permutation matrix (stream_shuffle on DVE
        # measures ~2.4us/call — far slower than this matmul)
        rps = ps.tile([128, 512], f32, tag="a", bufs=2, name="rps")
        nc.tensor.matmul(rps[:], rT_t[:], dst, start=True, stop=True)
        t1 = tmp.tile([128, 512], f32, tag="t1")
        nc.vector.tensor_mul(t1[:], rps[:], csn_t[:, tsl])   # PSUM x SBUF
        t2 = tmp.tile([128, 512], f32, tag="t2")
        nc.vector.tensor_mul(t2[:], dst, csc_t[:, tsl])      # f16 x f16
        nc.gpsimd.tensor_add(dst, t1[:], t2[:])              # SBUF-only

    def v_unit(tcc, b4):
        """V for s-chunk 4*tcc+b4, directly in [s, dh] layout."""
        i = 4 * tcc + b4
        pv = ps.tile([128, 512], f32, tag="a", bufs=2, name="pv")
        for kc in range(KC):
            nc.tensor.matmul(pv[:, 0:256],
                             xs(kc, tcc)[:, 128 * b4:128 * (b4 + 1)],
                             wv_t[kc][:],
                             start=(kc == 0), stop=(kc == KC - 1))
        vdst = v_all[:, 512 * i:512 * (i + 1)].rearrange(
            "p (a c) -> p a c", a=8, c=64)[:, 0::2, :]
        vsrc = pv[:, 0:256].rearrange("p (a c) -> p a c", a=4, c=64)
        if tcc == 0:
            nc.scalar.copy(vdst, vsrc)
        else:
            nc.vector.tensor_copy(vdst, vsrc)

    def proj_unit(j, oc, tag="a"):
        """One 128-row output-projection block (this core's partial)."""
        tsl = slice(512 * j, 512 * (j + 1))
        shape = [128, 1024] if tag == "s" else [128, 512]
        nb = 2 if tag in ("a", "s") else 1
        pp = ps.tile(shape, f32, tag=tag, bufs=nb, name="pp")[:, 0:512]
        # contract attn[1] first: on the last strip the final pair is p=0,
        # so its normalize gates only the second (stop) matmul
        for step, kc2 in enumerate((1, 0)):
            nc.tensor.matmul(
                pp, wp_t[kc2][:, 128 * oc:128 * (oc + 1)],
                attn[kc2][:, tsl],
                start=(step == 0), stop=(step == 1))
        base = 4096 * j + 512 * oc
        nc.vector.tensor_copy(obuf[:, base:base + 512], pp)
        oc_done[j] += 1
        # flush a strip when complete; the final strip flushes in halves
        # so the last DMA overlaps the last evictions
        if j == TC512 - 1:
            if oc_done[j] in (4, 8):
                lo = 4096 * j + (0 if oc_done[j] == 4 else 2048)
                out_flush(lo, lo + 2048)
        elif oc_done[j] == D // 128:
            out_flush(4096 * j, 4096 * (j + 1))

    # ===== filler queue: units pumped between attention score groups ====
    units = deque()   # entries: (cost_ns, kind, strip, fn)
    state = {"credit": 0}
    CAP = 2600

    def enqueue(kind, strip):
        if kind == "qk":
            for jc in range(4):
                units.append((2100, "qk", strip,
                              lambda jc=jc: qk_unit(strip, jc)))
        elif kind == "v":
            for b4 in range(4):
                units.append((950, "v", strip,
                              lambda b4=b4: v_unit(strip, b4)))
        else:
            for oc in range(D // 128):
                units.append((520, "proj", strip,
                              lambda oc=oc: proj_unit(strip, oc)))

    def pump(add):
        state["credit"] = min(CAP, state["credit"] + add)
        while units and state["credit"] >= units[0][0]:
            cost, _, _, fn = units.popleft()
            state["credit"] -= cost
            fn()

    def drain(kind, strip):
        keep = deque()
        while units:
            u = units.popleft()
            if u[1] == kind and u[2] <= strip:
                u[3]()
            else:
                keep.append(u)
        units.extend(keep)

    # ===== attention ====================================================
    def pair(j, p):
        """Causal attention for head pair p over query strip j."""
        drain("qk", j)
        pump(600)
        tsl = slice(512 * j, 512 * (j + 1))
        ni = 4 * (j + 1)
        qc = qk[2 * p]
        kch = qk[2 * p + 1]
        po = [ps.tile([128, 512], f32, tag=f"o{hh}", name=f"po{hh}")
              for hh in range(2)]

        def av(ptl, g):
            """A@V for one score group (emitted a few groups late)."""
            for hh in range(2):
                h = 2 * p + hh
                for half in range(2):
                    ii = 2 * g + half
                    diag = ii >= 4 * j
                    r = ii - 4 * j if diag else 0
                    c0 = 128 * r if diag else 0
                    nc.tensor.matmul(
                        po[hh][:, c0:512],
                        v_all[:, 512 * ii + 128 * h:512 * ii + 128 * (h + 1)],
                        ptl[hh][:, 512 * half + c0:512 * half + 512],
                        start=(ii == 0), stop=(ii == ni - 1))

        pending = []
        for g in range(ni // 2):
            ii0, ii1 = 2 * g, 2 * g + 1
            pss = [ps.tile([128, 1024], f32, tag="s", bufs=2,
                           name=f"pss{hh}") for hh in range(2)]
            for half in range(2):
                ii = 2 * g + half
                diag = ii >= 4 * j
                r = ii - 4 * j if diag else 0
                c0 = 512 * half + (128 * r if diag else 0)
                qs = slice(512 * j + 128 * r, 512 * (j + 1)) \
                    if diag else tsl
                for hh in range(2):
                    hsl = slice(64 * hh, 64 * (hh + 1))
                    nc.tensor.matmul(
                        pss[hh][:, c0:512 * half + 512],
                        kch[hsl, 128 * ii:128 * (ii + 1)],
                        qc[hsl, qs],
                        start=True, stop=True)
            ptl = [ptp.tile([128, 1024], f16, tag=f"ptl{hh}",
                            name=f"ptl{hh}") for hh in range(2)]
            # exp eviction; on late-diagonal groups only the live columns
            # are exp'd (two trimmed activations beat one full-width one)
            r1 = ii1 - 4 * j
            for hh in range(2):
                if ii0 >= 4 * j and r1 >= 2:
                    r0 = ii0 - 4 * j
                    nc.scalar.activation(
                        ptl[hh][:, 128 * r0:512], pss[hh][:, 128 * r0:512],
                        EXP, scale=0.125)
                    nc.scalar.activation(
                        ptl[hh][:, 512 + 128 * r1:1024],
                        pss[hh][:, 512 + 128 * r1:1024], EXP, scale=0.125)
                else:
                    nc.scalar.activation(ptl[hh][:], pss[hh][:], EXP,
                                         scale=0.125)
            # zero the in-block upper triangle of diagonal blocks (GpSimd;
            # columns past the 128-wide block are already all-keep)
            for half in range(2):
                ii = 2 * g + half
                if ii >= 4 * j:
                    r = ii - 4 * j
                    for hh in range(2):
                        reg = ptl[hh][:, 512 * half + 128 * r:
                                      512 * half + 128 * (r + 1)]
                        nc.gpsimd.tensor_mul(reg, reg, tri_t[:])
            pending.append((ptl, g))
            if len(pending) > 2:
                av(*pending.pop(0))
            pump(1000)
        drain("v", j)
        for pe_ in pending:
            pump(700)
            av(*pe_)

        for hh in range(2):
            sr = small.tile([64, 512], f32, tag="sr")
            nc.vector.tensor_copy(sr[:], po[hh][64:128, :])
            rc = small.tile([64, 512], f32, tag="rc")
            nc.vector.reciprocal_approx_fast(rc[:], sr[:])
            nc.vector.tensor_mul(
                attn[p][64 * hh:64 * (hh + 1), tsl],
                po[hh][0:64, :], rc[:])

    # ===== schedule =====================================================
    for jc in range(4):
        qk_unit(0, jc)
    for b4 in range(4):
        v_unit(0, b4)
    enqueue("qk", 1)
    enqueue("v", 1)
    for j in range(TC512):
        # on the last strip run pair 1 first so the tail-gating normalize
        # belongs to pair 0, whose attn is the LAST proj contraction step
        order = (1, 0) if j == TC512 - 1 else (0, 1)
        pair(j, order[0])
        pair(j, order[1])
        if j + 2 < TC512:
            enqueue("qk", j + 2)
            enqueue("v", j + 2)
        if j < TC512 - 1:
            enqueue("proj", j)
    # final drain: the last strip's projections run with accumulators
    # spread over every freed PSUM bank so the tail is PE-paced, with
    # output DMAs spread over three queues
    tags = ["a", "s", "o0", "o1", "a", "s"]
    while units:
        units.popleft()[3]()
    for oc in range(D // 128):
        proj_unit(TC512 - 1, oc, tag=tags[oc % len(tags)])


def _build_module():
    nc = bacc.Bacc("TRN2", target_bir_lowering=False, debug=False,
                   enable_asserts=False)
    d = {
        "xT2": nc.dram_tensor("xT2", [128, KC * T], f16,
                              kind="ExternalInput").ap(),
        "wqk2": nc.dram_tensor("wqk2", [128, KC * 512], f16,
                               kind="ExternalInput").ap(),
        "wv2": nc.dram_tensor("wv2", [128, KC * 256], f16,
                              kind="ExternalInput").ap(),
        "wp2": nc.dram_tensor("wp2", [128, 2 * D], f16,
                              kind="ExternalInput").ap(),
        "rT": nc.dram_tensor("rT", [128, 128], f16, kind="ExternalInput").ap(),
        "csc": nc.dram_tensor("csc", [128, T], f16, kind="ExternalInput").ap(),
        "csn": nc.dram_tensor("csn", [128, T], f16, kind="ExternalInput").ap(),
        "tri": nc.dram_tensor("tri", [128, 512], f16,
                              kind="ExternalInput").ap(),
        "o2": nc.dram_tensor("o2", [128, KC * T], f16,
                             kind="ExternalOutput").ap(),
    }
    with tile.TileContext(nc) as tc:
        with ExitStack() as ctx, \
             nc.allow_low_precision("fp16 PE operands are rounded by design"):
            _emit(nc, tc, d, ctx)
    nc.compile()
    return nc


def _get_module():
    if "nc" not in _CACHE:
        _CACHE["nc"] = _build_module()
    return _CACHE["nc"]


def _canonical(attn_mask, key_padding_mask):
    if attn_mask.shape != (1, 1, T, T) or key_padding_mask.shape != (B, T):
        return False
    if not key_padding_mask.all():
        return False
    m = np.asarray(attn_mask[0, 0], dtype=np.float32)
    causal = np.triu(np.full((T, T), -1e9, dtype=np.float32), k=1)
    return np.array_equal(m, causal)


def _reference_fallback(x, attn_mask, key_padding_mask, Wqkv, Wproj):
    x = np.asarray(x, np.float32)
    qkv = x @ np.asarray(Wqkv, np.float32).T
    q, k, v = qkv[..., :D], qkv[..., D:2 * D], qkv[..., 2 * D:]

    def split(t):
        return t.reshape(B, -1, H, DH).transpose(0, 2, 1, 3)

    def rope(xx):
        inv = 1.0 / (10000.0 ** (np.arange(0, DH, 2, dtype=np.float32) / DH))
        fr = np.arange(T, dtype=np.float32)[:, None] * inv[None, :]
        emb = np.concatenate([fr, fr], axis=-1)
        cos, sin = np.cos(emb)[None, None], np.sin(emb)[None, None]
        x1, x2 = xx[..., ::2], xx[..., 1::2]
        rh = np.stack((-x2, x1), axis=-1).reshape(xx.shape)
        return xx * cos + rh * sin

    q, k, v = split(q), split(k), split(v)
    q, k = rope(q), rope(k)
    s = np.einsum("bhtd,bhsd->bhts", q, k) / np.sqrt(np.float32(DH))
    s = s + np.asarray(attn_mask, np.float32)
    s = np.where(np.asarray(key_padding_mask)[:, None, None, :], s, -1e9)
    s = s - s.max(axis=-1, keepdims=True)
    e = np.exp(s)
    a = e / e.sum(axis=-1, keepdims=True)
    out = np.einsum("bhts,bhsd->bhtd", a, v)
    out = out.transpose(0, 2, 1, 3).reshape(B, T, D)
    return out @ np.asarray(Wproj, np.float32).T


def _make_in_maps(x, Wqkv, Wproj):
    csc, csn = _rope_tables()
    rT = _rot_matrix().astype(np.float16)
    tri = _tri_mask()

    Wq = np.asarray(Wqkv[:D], np.float32).reshape(H, DH, D)
    Wk = np.asarray(Wqkv[D:2 * D], np.float32).reshape(H, DH, D)
    Wv = np.asarray(Wqkv[2 * D:], np.float32).reshape(H, DH, D)
    WpT = np.ascontiguousarray(np.asarray(Wproj, np.float32).T)  # [din, dout]

    def pack(a):
        """[KC*128, n] -> [128, KC*n]: kc row-blocks become column blocks
        so every DRAM row is one fully contiguous partition line."""
        kc = a.shape[0] // 128
        return np.ascontiguousarray(
            a.reshape(kc, 128, a.shape[1]).transpose(1, 0, 2)
            .reshape(128, kc * a.shape[1]))

    def pack_chunks(a):
        """x^T [D, T] -> [128, 4*(8*512)]: per 512-col t-chunk, kc row
        blocks become column blocks (layout [chunk][kc][512])."""
        return np.ascontiguousarray(np.concatenate(
            [pack(a[:, 512 * c:512 * (c + 1)]) for c in range(TC512)],
            axis=1))

    xT2 = [pack_chunks(np.asarray(x[b], np.float32).T.astype(np.float16))
           for b in range(B)]

    in_maps = []
    for c in range(NCORES):
        b, g = divmod(c, GROUPS)
        hs = [HPC * g + hl for hl in range(HPC)]  # global head ids
        cols = []
        for pp in range(NPAIR):
            h0, h1 = hs[2 * pp], hs[2 * pp + 1]
            cols.append(np.concatenate([Wq[h0], Wq[h1]], axis=0))  # [128, D]
            cols.append(np.concatenate([Wk[h0], Wk[h1]], axis=0))
        wqk = np.ascontiguousarray(
            np.concatenate(cols, axis=0).T.astype(np.float16))     # [D, 512]
        wv = np.ascontiguousarray(
            np.concatenate([Wv[h] for h in hs], axis=0).T.astype(np.float16))
        wp = np.ascontiguousarray(
            WpT[256 * g:256 * (g + 1), :].astype(np.float16))  # [256, D]
        in_maps.append({
            "xT2": xT2[b], "wqk2": pack(wqk), "wv2": pack(wv),
            "wp2": pack(wp),
            "csc": csc, "csn": csn, "rT": rT, "tri": tri,
        })
    return in_maps


def _in_maps_for_trace(inputs):
    return _make_in_maps(np.asarray(inputs["x"]), np.asarray(inputs["Wqkv"]),
                         np.asarray(inputs["Wproj"]))


def kernel(x, attn_mask, key_padding_mask, Wqkv, Wproj):
    x = np.asarray(x)
    attn_mask = np.asarray(attn_mask)
    key_padding_mask = np.asarray(key_padding_mask)
    Wqkv = np.asarray(Wqkv)
    Wproj = np.asarray(Wproj)

    if not _canonical(attn_mask, key_padding_mask):
        return _reference_fallback(x, attn_mask, key_padding_mask, Wqkv, Wproj)

    nc = _get_module()
    in_maps = _make_in_maps(x, Wqkv, Wproj)
    res = bass_utils.run_bass_kernel_spmd(nc, in_maps,
                                          core_ids=list(range(NCORES)))
    out = np.empty((B, T, D), dtype=np.float32)
    for b in range(B):
        acc = res.results[4 * b]["o2"].astype(np.float32)
        for g in range(1, GROUPS):
            acc += res.results[4 * b + g]["o2"].astype(np.float32)
        # [128, 4 strips x 8 oc x 512] -> [T, D]
        y = acc.reshape(128, TC512, KC, 512)
        out[b] = y.transpose(1, 3, 2, 0).reshape(T, D)
    return out
